# revision 14
# baseline (speedup 1.0000x reference)
"""NeuroSAT-style GNN message passing on 8 Trainium2 NeuronCores.

Strategy (graph-level data parallelism): 32 graphs are sharded 4-per-core.
Per core, node states live in SBUF feature-major ([D=128 partitions, tokens]);
sparse message passing (gather + segment-sum) is done with SWDGE dma_gather
from token-major HBM tables, with the clause->literal segment-sum turned into
dense prefix-aligned gather planes by renumbering literals in descending
degree order (host-side).  All compute is fp32.
"""

import numpy as np
from contextlib import ExitStack

import concourse.bacc as bacc
import concourse.tile as tile
import concourse.mybir as mybir
from concourse import bass_utils, library_config

F32 = mybir.dt.float32
F32R = mybir.dt.float32r
I16 = mybir.dt.int16
AF = mybir.ActivationFunctionType
OP = mybir.AluOpType

D = 128
B = 32
NV = 500
NL_G = 2 * NV          # 1000 literals per graph
NC_G = 2000            # clauses per graph
K = 3
N_CORES = 8
GPC = B // N_CORES     # 4 graphs per core
NLc = GPC * NL_G       # 4000 literals per core
NCc = GPC * NC_G       # 8000 clauses per core
LPAD = 4096            # literal tokens padded (32 blocks of 128)
CPAD = 8064            # clause tokens padded (63 blocks of 128)
LT = LPAD + 1          # h_l table rows (row 0 = zeros)
CT = CPAD + 1          # h_c table rows (row 0 = zeros)
GCHUNK = 1024          # max indices per dma_gather call (SWDGE ring limit)

_prog_cache = {}


def _fmt_idx(idx):
    """int idx array [n] (n % 16 == 0) -> wrapped [128, n//16] int16 buffer."""
    n = len(idx)
    arr = np.asarray(idx, np.int16).reshape(n // 16, 16).T
    return np.tile(arr, (8, 1)).copy()


def _build_program(num_iters, d2_plane_sizes):
    """Build the SPMD Bacc program.  d2_plane_sizes: per-plane token counts
    (each a multiple of 128), shared across cores."""
    T2 = sum(d2_plane_sizes)
    S2 = T2 // 16

    nc = bacc.Bacc("TRN2", num_devices=N_CORES, debug=False,
                   target_bir_lowering=False)

    def din(name, shape, dtype=F32):
        return nc.dram_tensor(name, shape, dtype, kind="ExternalInput").ap()

    xl_feat = din("xl_feat", (D, LPAD))
    x_tok = din("x_tok", (LT, D))
    hc0 = din("hc0", (D, 1))
    w_ih_lc = din("w_ih_lc", (D, 4 * D))
    w_hh_lc = din("w_hh_lc", (D, 4 * D))
    w_ih_cl_m = din("w_ih_cl_m", (D, 4 * D))
    w_ih_cl_f = din("w_ih_cl_f", (D, 4 * D))
    w_hh_cl = din("w_hh_cl", (D, 4 * D))
    b_lc = din("b_lc", (D, 4))
    b_cl = din("b_cl", (D, 4))
    ind_d = din("ind", (D, (LPAD // D) * 4))
    out_w_b = din("out_w_b", (4, D))
    out_w_col = din("out_w_col", (D, 1))
    out_b = din("out_b", (1, 1))
    out_b4 = din("out_b4", (4, 1))
    ident = din("identity", (D, D))
    d1_idx_d = din("d1_idx", (128, 3 * CPAD // 16), I16)
    d2_idx_d = din("d2_idx", (128, S2), I16)
    flip_idx_d = din("flip_idx", (128, LPAD // 16), I16)

    h_l_tab = nc.dram_tensor("h_l_tab", (LT, D), F32, kind="Internal").ap()
    h_c_tab = nc.dram_tensor("h_c_tab", (CT, D), F32, kind="Internal").ap()

    hl_out = nc.dram_tensor("hl_out", (D, LPAD), F32, kind="ExternalOutput").ap()
    votes_out = nc.dram_tensor("votes_out", (1, LPAD), F32,
                               kind="ExternalOutput").ap()
    vr_out = nc.dram_tensor("vr_out", (4, 1), F32, kind="ExternalOutput").ap()

    NTILE_C, CW = 16, CPAD // 16     # clause token tiles of 504
    NTILE_L, LW = 8, LPAD // 8       # literal token tiles of 512

    with tile.TileContext(nc) as tc, ExitStack() as es:
        per = es.enter_context(tc.tile_pool(name="persist", bufs=1))
        h_l = per.tile(shape=[D, LPAD], dtype=F32)
        c_l = per.tile(shape=[D, LPAD], dtype=F32)
        h_c = per.tile(shape=[D, CPAD], dtype=F32)
        c_c = per.tile(shape=[D, CPAD], dtype=F32)
        union = per.tile(shape=[D, 2 * LPAD], dtype=F32)
        msg_c = union[:, :CPAD]
        msg_l = union[:, :LPAD]
        flip_b = union[:, LPAD:]
        w1 = per.tile(shape=[D, 4 * D], dtype=F32)
        w2 = per.tile(shape=[D, 4 * D], dtype=F32)
        w3 = per.tile(shape=[D, 4 * D], dtype=F32)
        w4 = per.tile(shape=[D, 4 * D], dtype=F32)
        w5 = per.tile(shape=[D, 4 * D], dtype=F32)
        blc = per.tile(shape=[D, 4], dtype=F32)
        bcl = per.tile(shape=[D, 4], dtype=F32)
        inds = per.tile(shape=[D, (LPAD // D) * 4], dtype=F32)
        owb = per.tile(shape=[4, D], dtype=F32)
        owc = per.tile(shape=[D, 1], dtype=F32)
        obs = per.tile(shape=[1, 1], dtype=F32)
        ob4 = per.tile(shape=[4, 1], dtype=F32)
        idm = per.tile(shape=[D, D], dtype=F32)
        hc0s = per.tile(shape=[D, 1], dtype=F32)
        d1_idx = per.tile(shape=[128, 3 * CPAD // 16], dtype=I16)
        d2_idx = per.tile(shape=[128, S2], dtype=I16)
        flip_idx = per.tile(shape=[128, LPAD // 16], dtype=I16)
        zrow = per.tile(shape=[1, D], dtype=F32)
        rvr = per.tile(shape=[4, 1], dtype=F32)
        vr_sb = per.tile(shape=[4, 1], dtype=F32)
        svr = per.tile(shape=[4, D], dtype=F32)

        gsc = es.enter_context(tc.tile_pool(name="gscratch", bufs=4))
        nc.gpsimd.load_library(library_config.mlp)

        for sb, dr in [(blc, b_lc), (bcl, b_cl), (inds, ind_d),
                       (owb, out_w_b), (obs, out_b),
                       (ob4, out_b4), (idm, ident), (hc0s, hc0),
                       (d1_idx, d1_idx_d), (d2_idx, d2_idx_d),
                       (flip_idx, flip_idx_d)]:
            nc.sync.dma_start(sb[:], dr)

        nc.vector.memset(c_l[:], 0.0)
        nc.vector.memset(c_c[:], 0.0)
        nc.vector.memset(h_c[:], 0.0)
        nc.vector.tensor_scalar_add(h_c[:], h_c[:],
                                    hc0s[:, 0:1])
        wstage = per.tile(shape=[D, 4 * D], dtype=F32)
        for wt, dr in [(w1, w_ih_lc), (w2, w_hh_lc), (w3, w_ih_cl_m),
                       (w4, w_ih_cl_f), (w5, w_hh_cl)]:
            nc.sync.dma_start(wstage[:], dr)
            nc.vector.tensor_copy(wt[:], wstage[:])
        nc.sync.dma_start(wstage[:, 0:1], out_w_col)
        nc.vector.tensor_copy(owc[:], wstage[:, 0:1])
        for a in range(0, LPAD, GCHUNK):
            xsc = gsc.tile(shape=[D, GCHUNK], dtype=F32, name="gchunk")
            nc.sync.dma_start(xsc[:], xl_feat[:, a:a + GCHUNK])
            nc.vector.tensor_copy(h_l[:, a:a + GCHUNK],
                                  xsc[:])
        nc.vector.memset(zrow[:], 0.0)
        nc.sync.dma_start(h_l_tab[0:1, :], zrow[:])
        nc.sync.dma_start(h_c_tab[0:1, :], zrow[:])

        bank = es.enter_context(tc.tile_pool(name="bank", bufs=7,
                                             space="PSUM"))
        psv = es.enter_context(tc.tile_pool(name="psv", bufs=1, space="PSUM"))
        tmp = es.enter_context(tc.tile_pool(name="tmp", bufs=3))
        stg = es.enter_context(tc.tile_pool(name="stage", bufs=4))

        def gather_tr(tab_ap, idx_sb, acc, base, src_t0, n, first):
            """Gather n tokens (idx positions [src_t0, src_t0+n)), transpose
            to feature-major, and copy (first=True) or add into acc columns
            [base, base+n)."""
            t = 0
            while t < n:
                L = min(GCHUNK, n - t)
                st0 = src_t0 + t
                sc = gsc.tile(shape=[D, GCHUNK], dtype=F32, name="gchunk")
                nc.gpsimd.dma_gather(
                    sc[:, :L].rearrange("p (b e) -> p b e", e=D), tab_ap,
                    idx_sb[:, st0 // 16:(st0 + L) // 16], L, L, D)
                for g0 in range(0, L // D, 4):
                    gn = min(4, L // D - g0)
                    pt = bank.tile(shape=[D, 4 * D], dtype=F32, name="bk")
                    for j in range(gn):
                        nc.tensor.transpose(
                            pt[:, j * D:(j + 1) * D],
                            sc[:, (g0 + j) * D:(g0 + j + 1) * D], idm[:])
                    cols = acc[:, base + t + g0 * D:
                               base + t + (g0 + gn) * D]
                    if first:
                        nc.vector.tensor_copy(cols,
                                              pt[:, :gn * D])
                    else:
                        nc.vector.tensor_tensor(cols, cols,
                                                pt[:, :gn * D], op=OP.add)
                t += L

        def transpose_to_table(src, nblocks, tab, row0):
            for g0 in range(0, nblocks, 4):
                gn = min(4, nblocks - g0)
                pt = bank.tile(shape=[D, 4 * D], dtype=F32, name="bk")
                st = stg.tile(shape=[D, 4 * D], dtype=F32, name="stt")
                for j in range(gn):
                    t = g0 + j
                    nc.tensor.transpose(pt[:, j * D:(j + 1) * D],
                                        src[:, t * D:(t + 1) * D], idm[:])
                nc.vector.tensor_copy(st[:, :gn * D], pt[:, :gn * D])
                dst = tab[row0 + g0 * D: row0 + (g0 + gn) * D, :].rearrange(
                    "(b p) e -> p b e", p=D)
                nc.sync.dma_start(dst, st[:, :gn * D].rearrange(
                    "p (b e) -> p b e", e=D))

        def lstm_side(n_tiles, width, x_terms, h_sb, c_sb, bias):
            for t in range(n_tiles):
                a, b2 = t * width, (t + 1) * width
                pg = [bank.tile(shape=[D, 4 * D], dtype=F32,
                                name="bk")[:, :width] for g in range(4)]
                for g in range(4):
                    for qi, (w_sb, rhs) in enumerate(x_terms):
                        nc.tensor.matmul(
                            pg[g][:],
                            w_sb[:, g * D:(g + 1) * D],
                            rhs[:, a:b2],
                            start=(qi == 0), stop=(qi == len(x_terms) - 1))
                nc.scalar.activation(pg[0][:], pg[0][:], AF.Sigmoid,
                                     bias=blc_or(bias, 0))
                nc.scalar.activation(pg[1][:], pg[1][:], AF.Sigmoid,
                                     bias=blc_or(bias, 1))
                tg = tmp.tile(shape=[D, width], dtype=F32, name="tg")
                nc.scalar.activation(tg[:], pg[2][:], AF.Tanh,
                                     bias=blc_or(bias, 2))
                nc.scalar.activation(pg[3][:], pg[3][:], AF.Sigmoid,
                                     bias=blc_or(bias, 3))
                t1 = tmp.tile(shape=[D, width], dtype=F32, name="t1")
                nc.vector.tensor_tensor(t1[:], pg[0][:], tg[:], op=OP.mult)
                nc.vector.tensor_tensor(c_sb[:, a:b2], c_sb[:, a:b2],
                                        pg[1][:], op=OP.mult)
                nc.vector.tensor_tensor(c_sb[:, a:b2], c_sb[:, a:b2], t1[:],
                                        op=OP.add)
                t2 = tmp.tile(shape=[D, width], dtype=F32, name="t2")
                nc.scalar.activation(t2[:], c_sb[:, a:b2], AF.Tanh)
                nc.vector.tensor_tensor(h_sb[:, a:b2],
                                        pg[3][:], t2[:], op=OP.mult)

        def blc_or(bias, g):
            return bias[:, g:g + 1]

        for it in range(num_iters):
            src_tab = x_tok if it == 0 else h_l_tab

            # direction 1: literal -> clause messages (3 slot planes)
            gather_tr(src_tab, d1_idx, union, 0, 0, CPAD, True)
            gather_tr(src_tab, d1_idx, union, 0, CPAD, CPAD, False)
            gather_tr(src_tab, d1_idx, union, 0, 2 * CPAD, CPAD, False)

            # clause LSTM
            lstm_side(NTILE_C, CW, [(w2, h_c), (w1, msg_c)], h_c, c_c, blc)

            # h_c -> token-major HBM table
            transpose_to_table(h_c, CPAD // D, h_c_tab, 1)

            # direction 2: clause -> literal messages (degree-prefix planes)
            gather_tr(h_c_tab, d2_idx, union, 0, 0, d2_plane_sizes[0], True)
            off = d2_plane_sizes[0]
            for n in d2_plane_sizes[1:]:
                gather_tr(h_c_tab, d2_idx, union, 0, off, n, False)
                off += n

            # flip: h_l[flip_perm]
            gather_tr(src_tab, flip_idx, union, LPAD, 0, LPAD, True)

            # literal LSTM
            lstm_side(NTILE_L, LW, [(w5, h_l), (w4, flip_b), (w3, msg_l)],
                      h_l, c_l, bcl)

            if it < num_iters - 1:
                transpose_to_table(h_l, LPAD // D, h_l_tab, 1)

        # readout: votes = out_w . h_l + out_b
        for t in range(NTILE_L):
            a, b2 = t * LW, (t + 1) * LW
            pv = bank.tile(shape=[D, 4 * D], dtype=F32, name="bk")[0:1, :LW]
            vst = stg.tile(shape=[1, LW], dtype=F32, name="vst")
            nc.tensor.matmul(pv[:], owc[:],
                             h_l[:, a:b2],
                             start=True, stop=True)
            nc.scalar.activation(vst[:], pv[:], AF.Identity,
                                 bias=obs[:, 0:1])
            nc.sync.dma_start(votes_out[:, a:b2], vst[:])
        nc.sync.dma_start(hl_out, h_l[:])

        # vote_reduced: indicator matmul over token-major h_l blocks
        pvr = psv.tile(shape=[4, D], dtype=F32, name="pvr")
        for t in range(LPAD // D):
            pt = bank.tile(shape=[D, 4 * D], dtype=F32, name="bk")[:, :D]
            st = stg.tile(shape=[D, D], dtype=F32, name="stv")
            nc.tensor.transpose(pt[:], h_l[:, t * D:(t + 1) * D], idm[:])
            nc.vector.tensor_copy(st[:], pt[:])
            nc.tensor.matmul(pvr[:], inds[:, t * 4:(t + 1) * 4], st[:],
                             start=(t == 0), stop=(t == LPAD // D - 1),
                             skip_group_check=True)
        nc.vector.tensor_tensor(svr[:], pvr[:], owb[:], op=OP.mult)
        nc.vector.tensor_reduce(rvr[:], svr[:], mybir.AxisListType.X, OP.add)
        nc.scalar.activation(vr_sb[:], rvr[:], AF.Identity,
                             bias=ob4[:, 0:1], scale=1.0 / NL_G)
        nc.sync.dma_start(vr_out, vr_sb[:])

    nc.compile()
    return nc


def _preprocess(inputs):
    """Slice per-core, degree-sort literals, build index planes + tables."""
    lit_idx = np.asarray(inputs["lit_idx"])
    clause_idx = np.asarray(inputs["clause_idx"])
    flip_perm = np.asarray(inputs["flip_perm"])
    x_unk = np.asarray(inputs["x_unk"], np.float32)

    order = np.argsort(clause_idx, kind="stable")
    lit_by_clause = lit_idx[order].reshape(B * NC_G, K)

    cores = []
    for c in range(N_CORES):
        l0, c0 = c * NLc, c * NCc
        lc = lit_by_clause[c0:c0 + NCc] - l0          # [NCc, K] in [0, NLc)
        deg = np.bincount(lc.reshape(-1), minlength=NLc)
        perm = np.argsort(-deg, kind="stable")        # rank -> orig literal
        rank_of = np.empty(NLc, np.int64)
        rank_of[perm] = np.arange(NLc)
        d1 = np.zeros((K, CPAD), np.int64)
        d1[:, :NCc] = (rank_of[lc] + 1).T
        sorted_deg = deg[perm]
        eorder = np.argsort(rank_of[lc.reshape(-1)], kind="stable")
        cl_of_edge = np.repeat(np.arange(NCc), K)[eorder]
        starts = np.zeros(NLc + 1, np.int64)
        np.cumsum(sorted_deg, out=starts[1:])
        fl = flip_perm[l0:l0 + NLc] - l0
        fidx = np.zeros(LPAD, np.int64)
        fidx[:NLc] = rank_of[fl[perm]] + 1
        cores.append(dict(perm=perm, rank_of=rank_of, sorted_deg=sorted_deg,
                          cl_of_edge=cl_of_edge, starts=starts, d1=d1,
                          fidx=fidx))

    max_deg = int(max(co["sorted_deg"][0] for co in cores))
    plane_sizes = [LPAD]
    for p in range(1, max_deg):
        n = max(int((co["sorted_deg"] > p).sum()) for co in cores)
        plane_sizes.append(min(-(-n // 128) * 128, LPAD))

    per_core = []
    for c, co in enumerate(cores):
        d2 = np.zeros((sum(plane_sizes),), np.int64)
        off = 0
        for p, npl in enumerate(plane_sizes):
            nreal = int((co["sorted_deg"] > p).sum())
            idxs = co["cl_of_edge"][co["starts"][:nreal] + p] + 1
            d2[off:off + nreal] = idxs
            off += npl
        x_core = x_unk[c * NLc:(c + 1) * NLc][co["perm"]]
        xl_feat = np.zeros((D, LPAD), np.float32)
        xl_feat[:, :NLc] = x_core.T
        x_tok = np.zeros((LT, D), np.float32)
        x_tok[1:1 + NLc] = x_core
        indicator = np.zeros((LPAD, 4), np.float32)
        g_of = co["perm"] // NL_G
        indicator[np.arange(NLc), g_of] = 1.0
        ind_sb = np.zeros((D, (LPAD // D) * 4), np.float32)
        for t in range(LPAD // D):
            ind_sb[:, t * 4:(t + 1) * 4] = indicator[t * D:(t + 1) * D]
        per_core.append(dict(
            d1_idx=_fmt_idx(co["d1"].reshape(-1)),
            d2_idx=_fmt_idx(d2),
            flip_idx=_fmt_idx(co["fidx"]),
            xl_feat=xl_feat, x_tok=x_tok, ind=ind_sb,
            perm=co["perm"], rank_of=co["rank_of"]))
    return per_core, tuple(plane_sizes)


def kernel(**inputs):
    num_iters = int(inputs["num_iters"])
    per_core, plane_sizes = _preprocess(inputs)

    f32 = lambda k: np.ascontiguousarray(np.asarray(inputs[k], np.float32))
    out_w = f32("out_w").reshape(D, 1)
    out_b = float(np.asarray(inputs["out_b"]).reshape(-1)[0])
    w_ih_cl = f32("Wih_cl").T.copy()

    shared = dict(
        hc0=(f32("C_w") + f32("C_b")).reshape(D, 1),
        w_ih_lc=f32("Wih_lc").T.copy(),
        w_hh_lc=f32("Whh_lc").T.copy(),
        w_ih_cl_m=np.ascontiguousarray(w_ih_cl[:D]),
        w_ih_cl_f=np.ascontiguousarray(w_ih_cl[D:]),
        w_hh_cl=f32("Whh_cl").T.copy(),
        b_lc=(f32("bih_lc") + f32("bhh_lc")).reshape(4, D).T.copy(),
        b_cl=(f32("bih_cl") + f32("bhh_cl")).reshape(4, D).T.copy(),
        out_w_b=np.tile(out_w.reshape(1, D), (4, 1)).copy(),
        out_w_col=out_w.copy(),
        out_b=np.full((1, 1), out_b, np.float32),
        out_b4=np.full((4, 1), out_b, np.float32),
        identity=np.eye(D, dtype=np.float32),
    )

    key = (num_iters, plane_sizes)
    if key not in _prog_cache:
        _prog_cache[key] = _build_program(num_iters, plane_sizes)
    nc = _prog_cache[key]

    in_maps = []
    for c in range(N_CORES):
        pc = per_core[c]
        m = dict(shared)
        m.update(xl_feat=pc["xl_feat"], x_tok=pc["x_tok"], ind=pc["ind"],
                 d1_idx=pc["d1_idx"], d2_idx=pc["d2_idx"],
                 flip_idx=pc["flip_idx"])
        in_maps.append(m)

    res = bass_utils.run_bass_kernel_spmd(nc, in_maps,
                                          core_ids=list(range(N_CORES)))

    h_l = np.empty((B * NL_G, D), np.float32)
    votes = np.empty((B * NL_G, 1), np.float32)
    vote_reduced = np.empty((B, 1), np.float32)
    for c in range(N_CORES):
        r = res.results[c]
        pc = per_core[c]
        h_l[c * NLc:(c + 1) * NLc] = r["hl_out"][:, :NLc].T[pc["rank_of"]]
        votes[c * NLc:(c + 1) * NLc] = \
            r["votes_out"][0, :NLc].reshape(-1, 1)[pc["rank_of"]]
        vote_reduced[c * GPC:(c + 1) * GPC] = r["vr_out"]
    return vote_reduced, votes, h_l


# revision 20
# speedup vs baseline: 1.0274x; 1.0274x over previous
"""NeuroSAT-style GNN message passing on 8 Trainium2 NeuronCores.

Strategy (graph-level data parallelism): 32 graphs are sharded 4-per-core.
Per core, node states live in SBUF feature-major ([D=128 partitions, tokens]);
sparse message passing (gather + segment-sum) is done with SWDGE dma_gather
from token-major HBM tables, with the clause->literal segment-sum turned into
dense prefix-aligned gather planes by renumbering literals in descending
degree order (host-side).  All compute is fp32.
"""

import numpy as np
from contextlib import ExitStack

import concourse.bacc as bacc
import concourse.tile as tile
import concourse.mybir as mybir
from concourse import bass_utils, library_config

F32 = mybir.dt.float32
F32R = mybir.dt.float32r
I16 = mybir.dt.int16
AF = mybir.ActivationFunctionType
OP = mybir.AluOpType

D = 128
B = 32
NV = 500
NL_G = 2 * NV          # 1000 literals per graph
NC_G = 2000            # clauses per graph
K = 3
N_CORES = 8
GPC = B // N_CORES     # 4 graphs per core
NLc = GPC * NL_G       # 4000 literals per core
NCc = GPC * NC_G       # 8000 clauses per core
LPAD = 4096            # literal tokens padded (32 blocks of 128)
CPAD = 8192            # clause tokens padded (64 blocks of 128)
LT = LPAD + 1          # h_l table rows (row 0 = zeros)
CT = CPAD + 1          # h_c table rows (row 0 = zeros)
GCHUNK = 1024
TOKSUM = False          # max indices per dma_gather call (SWDGE ring limit)

_prog_cache = {}


def _fmt_idx(idx):
    """int idx array [n] (n % 16 == 0) -> wrapped [128, n//16] int16 buffer."""
    n = len(idx)
    arr = np.asarray(idx, np.int16).reshape(n // 16, 16).T
    return np.tile(arr, (8, 1)).copy()


def _build_program(num_iters, d2_plane_sizes):
    """Build the SPMD Bacc program.  d2_plane_sizes: per-plane token counts
    (each a multiple of 128), shared across cores."""
    T2 = sum(d2_plane_sizes)
    S2 = T2 // 16

    nc = bacc.Bacc("TRN2", num_devices=N_CORES, debug=False,
                   target_bir_lowering=False)

    def din(name, shape, dtype=F32):
        return nc.dram_tensor(name, shape, dtype, kind="ExternalInput").ap()

    xl_feat = din("xl_feat", (D, LPAD))
    x_tok = din("x_tok", (LT, D))
    hc0 = din("hc0", (D, 1))
    w_ih_lc = din("w_ih_lc", (D, 4 * D))
    w_hh_lc = din("w_hh_lc", (D, 4 * D))
    w_ih_cl_m = din("w_ih_cl_m", (D, 4 * D))
    w_ih_cl_f = din("w_ih_cl_f", (D, 4 * D))
    w_hh_cl = din("w_hh_cl", (D, 4 * D))
    b_lc = din("b_lc", (D, 4))
    b_cl = din("b_cl", (D, 4))
    ind_d = din("ind", (D, (LPAD // D) * 4))
    out_w_b = din("out_w_b", (4, D))
    out_w_col = din("out_w_col", (D, 1))
    out_b = din("out_b", (1, 1))
    out_b4 = din("out_b4", (4, 1))
    ident = din("identity", (D, D))
    d1_idx_d = din("d1_idx", (128, 3 * CPAD // 16), I16)
    d2_idx_d = din("d2_idx", (128, S2), I16)
    flip_idx_d = din("flip_idx", (128, LPAD // 16), I16)

    h_l_tab = nc.dram_tensor("h_l_tab", (LT, D), F32, kind="Internal").ap()
    h_c_tab = nc.dram_tensor("h_c_tab", (CT, D), F32, kind="Internal").ap()

    hl_out = nc.dram_tensor("hl_out", (D, LPAD), F32, kind="ExternalOutput").ap()
    votes_out = nc.dram_tensor("votes_out", (1, LPAD), F32,
                               kind="ExternalOutput").ap()
    vr_out = nc.dram_tensor("vr_out", (4, 1), F32, kind="ExternalOutput").ap()

    NTILE_C, CW = 16, CPAD // 16     # clause token tiles of 512
    NTILE_L, LW = 8, LPAD // 8       # literal token tiles of 512

    with tile.TileContext(nc) as tc, ExitStack() as es:
        per = es.enter_context(tc.tile_pool(name="persist", bufs=1))
        h_l = per.tile(shape=[D, LPAD], dtype=F32)
        c_l = per.tile(shape=[D, LPAD], dtype=F32)
        h_c = per.tile(shape=[D, CPAD], dtype=F32)
        c_c = per.tile(shape=[D, CPAD], dtype=F32)
        union = per.tile(shape=[D, 2 * LPAD], dtype=F32)
        msg_c = union[:, :CPAD]
        msg_l = union[:, :LPAD]
        flip_b = union[:, LPAD:]
        w1 = per.tile(shape=[D, 4 * D], dtype=F32)
        w2 = per.tile(shape=[D, 4 * D], dtype=F32)
        w3 = per.tile(shape=[D, 4 * D], dtype=F32)
        w4 = per.tile(shape=[D, 4 * D], dtype=F32)
        w5 = per.tile(shape=[D, 4 * D], dtype=F32)
        blc = per.tile(shape=[D, 4], dtype=F32)
        bcl = per.tile(shape=[D, 4], dtype=F32)
        inds = per.tile(shape=[D, (LPAD // D) * 4], dtype=F32)
        owb = per.tile(shape=[4, D], dtype=F32)
        owc = per.tile(shape=[D, 1], dtype=F32)
        obs = per.tile(shape=[1, 1], dtype=F32)
        ob4 = per.tile(shape=[4, 1], dtype=F32)
        idm = per.tile(shape=[D, D], dtype=F32)
        hc0s = per.tile(shape=[D, 1], dtype=F32)
        d1_idx = per.tile(shape=[128, 3 * CPAD // 16], dtype=I16)
        d2_idx = per.tile(shape=[128, S2], dtype=I16)
        flip_idx = per.tile(shape=[128, LPAD // 16], dtype=I16)
        zrow = per.tile(shape=[1, D], dtype=F32)
        rvr = per.tile(shape=[4, 1], dtype=F32)
        vr_sb = per.tile(shape=[4, 1], dtype=F32)
        svr = per.tile(shape=[4, D], dtype=F32)

        gsc = es.enter_context(tc.tile_pool(name="gscratch", bufs=4))
        bank = es.enter_context(tc.tile_pool(name="bank", bufs=7,
                                             space="PSUM"))
        psv = es.enter_context(tc.tile_pool(name="psv", bufs=1, space="PSUM"))
        tmp = es.enter_context(tc.tile_pool(name="tmp", bufs=3))
        stg = es.enter_context(tc.tile_pool(name="stage", bufs=4))
        nc.gpsimd.load_library(library_config.mlp)

        for sb, dr in [(blc, b_lc), (bcl, b_cl), (inds, ind_d),
                       (owb, out_w_b), (obs, out_b),
                       (ob4, out_b4), (idm, ident), (hc0s, hc0),
                       (d1_idx, d1_idx_d), (d2_idx, d2_idx_d),
                       (flip_idx, flip_idx_d)]:
            nc.sync.dma_start(sb[:], dr)

        nc.vector.memset(c_l[:], 0.0)
        nc.vector.memset(c_c[:], 0.0)
        nc.vector.memset(h_c[:], 0.0)
        nc.vector.tensor_scalar_add(h_c[:], h_c[:],
                                    hc0s[:, 0:1])
        for wt, dr in [(w1, w_ih_lc), (w2, w_hh_lc), (w3, w_ih_cl_m),
                       (w4, w_ih_cl_f), (w5, w_hh_cl)]:
            ws = stg.tile(shape=[D, 4 * D], dtype=F32, name="stt")
            nc.sync.dma_start(ws[:], dr)
            nc.vector.tensor_copy(wt[:], ws[:])
        ws = stg.tile(shape=[D, 4 * D], dtype=F32, name="stt")
        nc.sync.dma_start(ws[:, 0:1], out_w_col)
        nc.vector.tensor_copy(owc[:], ws[:, 0:1])
        for a in range(0, LPAD, GCHUNK):
            xsc = gsc.tile(shape=[D, GCHUNK], dtype=F32, name="gchunk")
            nc.sync.dma_start(xsc[:], xl_feat[:, a:a + GCHUNK])
            nc.vector.tensor_copy(h_l[:, a:a + GCHUNK],
                                  xsc[:])
        nc.vector.memset(zrow[:], 0.0)
        nc.sync.dma_start(h_l_tab[0:1, :], zrow[:])
        nc.sync.dma_start(h_c_tab[0:1, :], zrow[:])


        def gather_tr(tab_ap, idx_sb, acc, base, src_t0, n, first):
            """Gather n tokens (idx positions [src_t0, src_t0+n)), transpose
            to feature-major, and copy (first=True) or add into acc columns
            [base, base+n)."""
            t = 0
            while t < n:
                L = min(GCHUNK, n - t)
                st0 = src_t0 + t
                sc = gsc.tile(shape=[D, GCHUNK], dtype=F32, name="gchunk")
                nc.gpsimd.dma_gather(
                    sc[:, :L].rearrange("p (b e) -> p b e", e=D), tab_ap,
                    idx_sb[:, st0 // 16:(st0 + L) // 16], L, L, D)
                for g0 in range(0, L // D, 4):
                    gn = min(4, L // D - g0)
                    pt = bank.tile(shape=[D, 4 * D], dtype=F32, name="bk")
                    for j in range(gn):
                        nc.tensor.transpose(
                            pt[:, j * D:(j + 1) * D],
                            sc[:, (g0 + j) * D:(g0 + j + 1) * D], idm[:])
                    cols = acc[:, base + t + g0 * D:
                               base + t + (g0 + gn) * D]
                    if first:
                        nc.vector.tensor_copy(cols,
                                              pt[:, :gn * D])
                    else:
                        nc.vector.tensor_tensor(cols, cols,
                                                pt[:, :gn * D], op=OP.add)
                t += L

        def gather_tok(tab_ap, idx_sb, dst, base, src_t0, n, first):
            """Gather n tokens into token-major dst cols [base, base+n) (copy
            or DVE-add via scratch)."""
            t = 0
            while t < n:
                L = min(GCHUNK, n - t)
                st0 = src_t0 + t
                if first:
                    nc.gpsimd.dma_gather(
                        dst[:, base + t:base + t + L].rearrange(
                            "p (b e) -> p b e", e=D), tab_ap,
                        idx_sb[:, st0 // 16:(st0 + L) // 16], L, L, D)
                else:
                    sc = gsc.tile(shape=[D, GCHUNK], dtype=F32, name="gchunk")
                    nc.gpsimd.dma_gather(
                        sc[:, :L].rearrange("p (b e) -> p b e", e=D), tab_ap,
                        idx_sb[:, st0 // 16:(st0 + L) // 16], L, L, D)
                    a = dst[:, base + t:base + t + L]
                    nc.vector.tensor_tensor(a, a, sc[:, :L], op=OP.add)
                t += L

        def transpose_inplace(buf, base, nblocks):
            for g0 in range(0, nblocks, 4):
                gn = min(4, nblocks - g0)
                pt = bank.tile(shape=[D, 4 * D], dtype=F32, name="bk")
                for j in range(gn):
                    t = g0 + j
                    nc.tensor.transpose(
                        pt[:, j * D:(j + 1) * D],
                        buf[:, base + t * D:base + (t + 1) * D], idm[:])
                nc.vector.tensor_copy(
                    buf[:, base + g0 * D:base + (g0 + gn) * D],
                    pt[:, :gn * D])

        def transpose_to_table(src, b_lo, b_hi, tab, row0):
            for g0 in range(b_lo, b_hi, 4):
                gn = min(4, b_hi - g0)
                pt = bank.tile(shape=[D, 4 * D], dtype=F32, name="bk")
                st = stg.tile(shape=[D, 4 * D], dtype=F32, name="stt")
                for j in range(gn):
                    t = g0 + j
                    nc.tensor.transpose(pt[:, j * D:(j + 1) * D],
                                        src[:, t * D:(t + 1) * D], idm[:])
                nc.vector.tensor_copy(st[:, :gn * D], pt[:, :gn * D])
                dst = tab[row0 + g0 * D: row0 + (g0 + gn) * D, :].rearrange(
                    "(b p) e -> p b e", p=D)
                nc.sync.dma_start(dst, st[:, :gn * D].rearrange(
                    "p (b e) -> p b e", e=D))

        def lstm_side(t_lo, t_hi, width, x_terms, h_sb, c_sb, bias):
            for t in range(t_lo, t_hi):
                a, b2 = t * width, (t + 1) * width
                pg = [bank.tile(shape=[D, 4 * D], dtype=F32,
                                name="bk")[:, :width] for g in range(4)]
                for g in range(4):
                    for qi, (w_sb, rhs) in enumerate(x_terms):
                        nc.tensor.matmul(
                            pg[g][:],
                            w_sb[:, g * D:(g + 1) * D],
                            rhs[:, a:b2],
                            start=(qi == 0), stop=(qi == len(x_terms) - 1))
                nc.scalar.activation(pg[0][:], pg[0][:], AF.Sigmoid,
                                     bias=blc_or(bias, 0))
                nc.scalar.activation(pg[1][:], pg[1][:], AF.Sigmoid,
                                     bias=blc_or(bias, 1))
                tg = tmp.tile(shape=[D, width], dtype=F32, name="tg")
                nc.scalar.activation(tg[:], pg[2][:], AF.Tanh,
                                     bias=blc_or(bias, 2))
                nc.scalar.activation(pg[3][:], pg[3][:], AF.Sigmoid,
                                     bias=blc_or(bias, 3))
                t1 = tmp.tile(shape=[D, width], dtype=F32, name="t1")
                nc.vector.tensor_tensor(t1[:], pg[0][:], tg[:], op=OP.mult)
                nc.vector.tensor_tensor(c_sb[:, a:b2], c_sb[:, a:b2],
                                        pg[1][:], op=OP.mult)
                nc.vector.tensor_tensor(c_sb[:, a:b2], c_sb[:, a:b2], t1[:],
                                        op=OP.add)
                t2 = tmp.tile(shape=[D, width], dtype=F32, name="t2")
                nc.scalar.activation(t2[:], c_sb[:, a:b2], AF.Tanh)
                nc.vector.tensor_tensor(h_sb[:, a:b2],
                                        pg[3][:], t2[:], op=OP.mult)

        def blc_or(bias, g):
            return bias[:, g:g + 1]

        for it in range(num_iters):
            src_tab = x_tok if it == 0 else h_l_tab

            # --- direction 1 + clause LSTM, software-pipelined halves ---
            HC = CPAD // 2
            for a in (0, HC):
                gather_tr(src_tab, d1_idx, union, a, 0 * CPAD + a, HC, True)
                gather_tr(src_tab, d1_idx, union, a, 1 * CPAD + a, HC, False)
                gather_tr(src_tab, d1_idx, union, a, 2 * CPAD + a, HC, False)
                lstm_side(a // CW, (a + HC) // CW, CW,
                          [(w2, h_c), (w1, msg_c)], h_c, c_c, blc)
                transpose_to_table(h_c, a // D, (a + HC) // D, h_c_tab, 1)

            # --- direction 2 + flip + literal LSTM, pipelined halves ---
            HL = LPAD // 2
            gather_tr(src_tab, flip_idx, union, LPAD, 0, LPAD, True)
            for lo in (0, HL):
                hi = lo + HL
                first = True
                off = 0
                for n in d2_plane_sizes:
                    s0, s1 = lo, min(n, hi)
                    if s1 > s0:
                        gather_tr(h_c_tab, d2_idx, union, s0, off + s0,
                                  s1 - s0, first)
                    first = False
                    off += n
                lstm_side(lo // LW, hi // LW, LW,
                          [(w5, h_l), (w4, flip_b), (w3, msg_l)],
                          h_l, c_l, bcl)
                if it < num_iters - 1:
                    transpose_to_table(h_l, lo // D, hi // D, h_l_tab, 1)


        # readout: votes = out_w . h_l + out_b
        for t in range(NTILE_L):
            a, b2 = t * LW, (t + 1) * LW
            pv = bank.tile(shape=[D, 4 * D], dtype=F32, name="bk")[0:1, :LW]
            vst = stg.tile(shape=[1, LW], dtype=F32, name="vst")
            nc.tensor.matmul(pv[:], owc[:],
                             h_l[:, a:b2],
                             start=True, stop=True)
            nc.scalar.activation(vst[:], pv[:], AF.Identity,
                                 bias=obs[:, 0:1])
            nc.sync.dma_start(votes_out[:, a:b2], vst[:])
        nc.sync.dma_start(hl_out, h_l[:])

        # vote_reduced: indicator matmul over token-major h_l blocks
        pvr = psv.tile(shape=[4, D], dtype=F32, name="pvr")
        for t in range(LPAD // D):
            pt = bank.tile(shape=[D, 4 * D], dtype=F32, name="bk")[:, :D]
            st = stg.tile(shape=[D, D], dtype=F32, name="stv")
            nc.tensor.transpose(pt[:], h_l[:, t * D:(t + 1) * D], idm[:])
            nc.vector.tensor_copy(st[:], pt[:])
            nc.tensor.matmul(pvr[:], inds[:, t * 4:(t + 1) * 4], st[:],
                             start=(t == 0), stop=(t == LPAD // D - 1),
                             skip_group_check=True)
        nc.vector.tensor_tensor(svr[:], pvr[:], owb[:], op=OP.mult)
        nc.vector.tensor_reduce(rvr[:], svr[:], mybir.AxisListType.X, OP.add)
        nc.scalar.activation(vr_sb[:], rvr[:], AF.Identity,
                             bias=ob4[:, 0:1], scale=1.0 / NL_G)
        nc.sync.dma_start(vr_out, vr_sb[:])

    nc.compile()
    return nc


def _preprocess(inputs):
    """Slice per-core, degree-sort literals, build index planes + tables."""
    lit_idx = np.asarray(inputs["lit_idx"])
    clause_idx = np.asarray(inputs["clause_idx"])
    flip_perm = np.asarray(inputs["flip_perm"])
    x_unk = np.asarray(inputs["x_unk"], np.float32)

    order = np.argsort(clause_idx, kind="stable")
    lit_by_clause = lit_idx[order].reshape(B * NC_G, K)

    cores = []
    for c in range(N_CORES):
        l0, c0 = c * NLc, c * NCc
        lc = lit_by_clause[c0:c0 + NCc] - l0          # [NCc, K] in [0, NLc)
        deg = np.bincount(lc.reshape(-1), minlength=NLc)
        perm = np.argsort(-deg, kind="stable")        # rank -> orig literal
        rank_of = np.empty(NLc, np.int64)
        rank_of[perm] = np.arange(NLc)
        d1 = np.zeros((K, CPAD), np.int64)
        d1[:, :NCc] = (rank_of[lc] + 1).T
        sorted_deg = deg[perm]
        eorder = np.argsort(rank_of[lc.reshape(-1)], kind="stable")
        cl_of_edge = np.repeat(np.arange(NCc), K)[eorder]
        starts = np.zeros(NLc + 1, np.int64)
        np.cumsum(sorted_deg, out=starts[1:])
        fl = flip_perm[l0:l0 + NLc] - l0
        fidx = np.zeros(LPAD, np.int64)
        fidx[:NLc] = rank_of[fl[perm]] + 1
        cores.append(dict(perm=perm, rank_of=rank_of, sorted_deg=sorted_deg,
                          cl_of_edge=cl_of_edge, starts=starts, d1=d1,
                          fidx=fidx))

    max_deg = int(max(co["sorted_deg"][0] for co in cores))
    plane_sizes = [LPAD]
    for p in range(1, max_deg):
        n = max(int((co["sorted_deg"] > p).sum()) for co in cores)
        plane_sizes.append(min(-(-n // 128) * 128, LPAD))

    per_core = []
    for c, co in enumerate(cores):
        d2 = np.zeros((sum(plane_sizes),), np.int64)
        off = 0
        for p, npl in enumerate(plane_sizes):
            nreal = int((co["sorted_deg"] > p).sum())
            idxs = co["cl_of_edge"][co["starts"][:nreal] + p] + 1
            d2[off:off + nreal] = idxs
            off += npl
        x_core = x_unk[c * NLc:(c + 1) * NLc][co["perm"]]
        xl_feat = np.zeros((D, LPAD), np.float32)
        xl_feat[:, :NLc] = x_core.T
        x_tok = np.zeros((LT, D), np.float32)
        x_tok[1:1 + NLc] = x_core
        indicator = np.zeros((LPAD, 4), np.float32)
        g_of = co["perm"] // NL_G
        indicator[np.arange(NLc), g_of] = 1.0
        ind_sb = np.zeros((D, (LPAD // D) * 4), np.float32)
        for t in range(LPAD // D):
            ind_sb[:, t * 4:(t + 1) * 4] = indicator[t * D:(t + 1) * D]
        per_core.append(dict(
            d1_idx=_fmt_idx(co["d1"].reshape(-1)),
            d2_idx=_fmt_idx(d2),
            flip_idx=_fmt_idx(co["fidx"]),
            xl_feat=xl_feat, x_tok=x_tok, ind=ind_sb,
            perm=co["perm"], rank_of=co["rank_of"]))
    return per_core, tuple(plane_sizes)


def kernel(**inputs):
    num_iters = int(inputs["num_iters"])
    per_core, plane_sizes = _preprocess(inputs)

    f32 = lambda k: np.ascontiguousarray(np.asarray(inputs[k], np.float32))
    out_w = f32("out_w").reshape(D, 1)
    out_b = float(np.asarray(inputs["out_b"]).reshape(-1)[0])
    w_ih_cl = f32("Wih_cl").T.copy()

    shared = dict(
        hc0=(f32("C_w") + f32("C_b")).reshape(D, 1),
        w_ih_lc=f32("Wih_lc").T.copy(),
        w_hh_lc=f32("Whh_lc").T.copy(),
        w_ih_cl_m=np.ascontiguousarray(w_ih_cl[:D]),
        w_ih_cl_f=np.ascontiguousarray(w_ih_cl[D:]),
        w_hh_cl=f32("Whh_cl").T.copy(),
        b_lc=(f32("bih_lc") + f32("bhh_lc")).reshape(4, D).T.copy(),
        b_cl=(f32("bih_cl") + f32("bhh_cl")).reshape(4, D).T.copy(),
        out_w_b=np.tile(out_w.reshape(1, D), (4, 1)).copy(),
        out_w_col=out_w.copy(),
        out_b=np.full((1, 1), out_b, np.float32),
        out_b4=np.full((4, 1), out_b, np.float32),
        identity=np.eye(D, dtype=np.float32),
    )

    key = (num_iters, plane_sizes)
    if key not in _prog_cache:
        _prog_cache[key] = _build_program(num_iters, plane_sizes)
    nc = _prog_cache[key]

    in_maps = []
    for c in range(N_CORES):
        pc = per_core[c]
        m = dict(shared)
        m.update(xl_feat=pc["xl_feat"], x_tok=pc["x_tok"], ind=pc["ind"],
                 d1_idx=pc["d1_idx"], d2_idx=pc["d2_idx"],
                 flip_idx=pc["flip_idx"])
        in_maps.append(m)

    res = bass_utils.run_bass_kernel_spmd(nc, in_maps,
                                          core_ids=list(range(N_CORES)))

    h_l = np.empty((B * NL_G, D), np.float32)
    votes = np.empty((B * NL_G, 1), np.float32)
    vote_reduced = np.empty((B, 1), np.float32)
    for c in range(N_CORES):
        r = res.results[c]
        pc = per_core[c]
        h_l[c * NLc:(c + 1) * NLc] = r["hl_out"][:, :NLc].T[pc["rank_of"]]
        votes[c * NLc:(c + 1) * NLc] = \
            r["votes_out"][0, :NLc].reshape(-1, 1)[pc["rank_of"]]
        vote_reduced[c * GPC:(c + 1) * GPC] = r["vr_out"]
    return vote_reduced, votes, h_l


# revision 24
# speedup vs baseline: 1.0349x; 1.0073x over previous
"""NeuroSAT-style GNN message passing on 8 Trainium2 NeuronCores.

Strategy (graph-level data parallelism): 32 graphs are sharded 4-per-core.
Per core, node states live in SBUF feature-major ([D=128 partitions, tokens]);
sparse message passing (gather + segment-sum) is done with SWDGE dma_gather
from token-major HBM tables, with the clause->literal segment-sum turned into
dense prefix-aligned gather planes by renumbering literals in descending
degree order (host-side).  All compute is fp32.
"""

import numpy as np
from contextlib import ExitStack

import concourse.bacc as bacc
import concourse.tile as tile
import concourse.mybir as mybir
from concourse import bass_utils, library_config

F32 = mybir.dt.float32
F32R = mybir.dt.float32r
I16 = mybir.dt.int16
AF = mybir.ActivationFunctionType
OP = mybir.AluOpType

D = 128
B = 32
NV = 500
NL_G = 2 * NV          # 1000 literals per graph
NC_G = 2000            # clauses per graph
K = 3
N_CORES = 8
GPC = B // N_CORES     # 4 graphs per core
NLc = GPC * NL_G       # 4000 literals per core
NCc = GPC * NC_G       # 8000 clauses per core
LPAD = 4096            # literal tokens padded (32 blocks of 128)
CPAD = 8192            # clause tokens padded (64 blocks of 128)
LT = LPAD + 1          # h_l table rows (row 0 = zeros)
CT = CPAD + 1          # h_c table rows (row 0 = zeros)
GCHUNK = 1024
TOKSUM = False          # max indices per dma_gather call (SWDGE ring limit)

_prog_cache = {}


def _fmt_idx(idx):
    """int idx array [n] (n % 16 == 0) -> wrapped [128, n//16] int16 buffer."""
    n = len(idx)
    arr = np.asarray(idx, np.int16).reshape(n // 16, 16).T
    return np.tile(arr, (8, 1)).copy()


def _build_program(num_iters, d2_plane_sizes):
    """Build the SPMD Bacc program.  d2_plane_sizes: per-plane token counts
    (each a multiple of 128), shared across cores."""
    T2 = sum(d2_plane_sizes)
    S2 = T2 // 16

    nc = bacc.Bacc("TRN2", num_devices=N_CORES, debug=False,
                   target_bir_lowering=False)

    def din(name, shape, dtype=F32):
        return nc.dram_tensor(name, shape, dtype, kind="ExternalInput").ap()

    xl_feat = din("xl_feat", (D, LPAD))
    x_tok = din("x_tok", (LT, D))
    hc0 = din("hc0", (D, 1))
    w_ih_lc = din("w_ih_lc", (D, 4 * D))
    w_hh_lc = din("w_hh_lc", (D, 4 * D))
    w_ih_cl_m = din("w_ih_cl_m", (D, 4 * D))
    w_ih_cl_f = din("w_ih_cl_f", (D, 4 * D))
    w_hh_cl = din("w_hh_cl", (D, 4 * D))
    b_lc = din("b_lc", (D, 4))
    b_cl = din("b_cl", (D, 4))
    ind_d = din("ind", (D, (LPAD // D) * 4))
    out_w_b = din("out_w_b", (4, D))
    out_w_col = din("out_w_col", (D, 1))
    out_b = din("out_b", (1, 1))
    out_b4 = din("out_b4", (4, 1))
    ident = din("identity", (D, D))
    d1_idx_d = din("d1_idx", (128, 3 * CPAD // 16), I16)
    d2_idx_d = din("d2_idx", (128, S2), I16)
    flip_idx_d = din("flip_idx", (128, LPAD // 16), I16)

    h_l_tab = nc.dram_tensor("h_l_tab", (LT, D), F32, kind="Internal").ap()
    h_c_tab = nc.dram_tensor("h_c_tab", (CT, D), F32, kind="Internal").ap()

    hl_out = nc.dram_tensor("hl_out", (D, LPAD), F32, kind="ExternalOutput").ap()
    votes_out = nc.dram_tensor("votes_out", (1, LPAD), F32,
                               kind="ExternalOutput").ap()
    vr_out = nc.dram_tensor("vr_out", (4, 1), F32, kind="ExternalOutput").ap()

    NTILE_C, CW = 16, CPAD // 16     # clause token tiles of 512
    NTILE_L, LW = 8, LPAD // 8       # literal token tiles of 512

    with tile.TileContext(nc) as tc, ExitStack() as es:
        per = es.enter_context(tc.tile_pool(name="persist", bufs=1))
        h_l = per.tile(shape=[D, LPAD], dtype=F32)
        c_l = per.tile(shape=[D, LPAD], dtype=F32)
        h_c = per.tile(shape=[D, CPAD], dtype=F32)
        c_c = per.tile(shape=[D, CPAD], dtype=F32)
        union = per.tile(shape=[D, 2 * LPAD], dtype=F32)
        msg_c = union[:, :CPAD]
        msg_l = union[:, :LPAD]
        flip_b = union[:, LPAD:]
        w1 = per.tile(shape=[D, 4 * D], dtype=F32)
        w2 = per.tile(shape=[D, 4 * D], dtype=F32)
        w3 = per.tile(shape=[D, 4 * D], dtype=F32)
        w4 = per.tile(shape=[D, 4 * D], dtype=F32)
        w5 = per.tile(shape=[D, 4 * D], dtype=F32)
        blc = per.tile(shape=[D, 4], dtype=F32)
        bcl = per.tile(shape=[D, 4], dtype=F32)
        inds = per.tile(shape=[D, (LPAD // D) * 4], dtype=F32)
        owb = per.tile(shape=[4, D], dtype=F32)
        owc = per.tile(shape=[D, 1], dtype=F32)
        obs = per.tile(shape=[1, 1], dtype=F32)
        ob4 = per.tile(shape=[4, 1], dtype=F32)
        idm = per.tile(shape=[D, D], dtype=F32)
        hc0s = per.tile(shape=[D, 1], dtype=F32)
        d1_idx = per.tile(shape=[128, 3 * CPAD // 16], dtype=I16)
        d2_idx = per.tile(shape=[128, S2], dtype=I16)
        flip_idx = per.tile(shape=[128, LPAD // 16], dtype=I16)
        zrow = per.tile(shape=[1, D], dtype=F32)
        rvr = per.tile(shape=[4, 1], dtype=F32)
        vr_sb = per.tile(shape=[4, 1], dtype=F32)
        svr = per.tile(shape=[4, D], dtype=F32)

        gsc = es.enter_context(tc.tile_pool(name="gscratch", bufs=4))
        bank = es.enter_context(tc.tile_pool(name="bank", bufs=7,
                                             space="PSUM"))
        psv = es.enter_context(tc.tile_pool(name="psv", bufs=1, space="PSUM"))
        tmp = es.enter_context(tc.tile_pool(name="tmp", bufs=3))
        stg = es.enter_context(tc.tile_pool(name="stage", bufs=4))
        nc.gpsimd.load_library(library_config.mlp)

        for sb, dr in [(d1_idx, d1_idx_d), (idm, ident),
                       (d2_idx, d2_idx_d), (flip_idx, flip_idx_d),
                       (hc0s, hc0), (blc, b_lc), (bcl, b_cl),
                       (inds, ind_d), (owb, out_w_b), (obs, out_b),
                       (ob4, out_b4)]:
            nc.sync.dma_start(sb[:], dr)

        nc.vector.memset(c_l[:], 0.0)
        nc.vector.memset(c_c[:], 0.0)
        nc.vector.memset(h_c[:], 0.0)
        nc.vector.tensor_scalar_add(h_c[:], h_c[:],
                                    hc0s[:, 0:1])
        for a in range(0, LPAD, 512):
            xsc = tmp.tile(shape=[D, 512], dtype=F32, name="t1")
            nc.sync.dma_start(xsc[:], xl_feat[:, a:a + 512])
            nc.vector.tensor_copy(h_l[:, a:a + 512], xsc[:])
        for wt, dr in [(w1, w_ih_lc), (w2, w_hh_lc), (w3, w_ih_cl_m),
                       (w4, w_ih_cl_f), (w5, w_hh_cl)]:
            ws = stg.tile(shape=[D, 4 * D], dtype=F32, name="stt")
            nc.sync.dma_start(ws[:], dr)
            nc.vector.tensor_copy(wt[:], ws[:])
        ws = stg.tile(shape=[D, 4 * D], dtype=F32, name="stt")
        nc.sync.dma_start(ws[:, 0:1], out_w_col)
        nc.vector.tensor_copy(owc[:], ws[:, 0:1])
        nc.vector.memset(zrow[:], 0.0)
        nc.sync.dma_start(h_l_tab[0:1, :], zrow[:])
        nc.sync.dma_start(h_c_tab[0:1, :], zrow[:])


        def gather_tr(tab_ap, idx_sb, acc, base, src_t0, n, first):
            """Gather n tokens (idx positions [src_t0, src_t0+n)), transpose
            to feature-major, and copy (first=True) or add into acc columns
            [base, base+n)."""
            t = 0
            while t < n:
                L = min(GCHUNK, n - t)
                st0 = src_t0 + t
                sc = gsc.tile(shape=[D, GCHUNK], dtype=F32, name="gchunk")
                nc.gpsimd.dma_gather(
                    sc[:, :L].rearrange("p (b e) -> p b e", e=D), tab_ap,
                    idx_sb[:, st0 // 16:(st0 + L) // 16], L, L, D)
                for g0 in range(0, L // D, 4):
                    gn = min(4, L // D - g0)
                    pt = bank.tile(shape=[D, 4 * D], dtype=F32, name="bk")
                    for j in range(gn):
                        nc.tensor.transpose(
                            pt[:, j * D:(j + 1) * D],
                            sc[:, (g0 + j) * D:(g0 + j + 1) * D], idm[:])
                    cols = acc[:, base + t + g0 * D:
                               base + t + (g0 + gn) * D]
                    if first:
                        nc.vector.tensor_copy(cols,
                                              pt[:, :gn * D])
                    else:
                        nc.vector.tensor_tensor(cols, cols,
                                                pt[:, :gn * D], op=OP.add)
                t += L

        def gather_tok(tab_ap, idx_sb, dst, base, src_t0, n, first):
            """Gather n tokens into token-major dst cols [base, base+n) (copy
            or DVE-add via scratch)."""
            t = 0
            while t < n:
                L = min(GCHUNK, n - t)
                st0 = src_t0 + t
                if first:
                    nc.gpsimd.dma_gather(
                        dst[:, base + t:base + t + L].rearrange(
                            "p (b e) -> p b e", e=D), tab_ap,
                        idx_sb[:, st0 // 16:(st0 + L) // 16], L, L, D)
                else:
                    sc = gsc.tile(shape=[D, GCHUNK], dtype=F32, name="gchunk")
                    nc.gpsimd.dma_gather(
                        sc[:, :L].rearrange("p (b e) -> p b e", e=D), tab_ap,
                        idx_sb[:, st0 // 16:(st0 + L) // 16], L, L, D)
                    a = dst[:, base + t:base + t + L]
                    nc.vector.tensor_tensor(a, a, sc[:, :L], op=OP.add)
                t += L

        def transpose_inplace(buf, base, nblocks):
            for g0 in range(0, nblocks, 4):
                gn = min(4, nblocks - g0)
                pt = bank.tile(shape=[D, 4 * D], dtype=F32, name="bk")
                for j in range(gn):
                    t = g0 + j
                    nc.tensor.transpose(
                        pt[:, j * D:(j + 1) * D],
                        buf[:, base + t * D:base + (t + 1) * D], idm[:])
                nc.vector.tensor_copy(
                    buf[:, base + g0 * D:base + (g0 + gn) * D],
                    pt[:, :gn * D])

        def transpose_to_table(src, b_lo, b_hi, tab, row0):
            for g0 in range(b_lo, b_hi, 4):
                gn = min(4, b_hi - g0)
                pt = bank.tile(shape=[D, 4 * D], dtype=F32, name="bk")
                st = stg.tile(shape=[D, 4 * D], dtype=F32, name="stt")
                for j in range(gn):
                    t = g0 + j
                    nc.tensor.transpose(pt[:, j * D:(j + 1) * D],
                                        src[:, t * D:(t + 1) * D], idm[:])
                nc.vector.tensor_copy(st[:, :gn * D], pt[:, :gn * D])
                dst = tab[row0 + g0 * D: row0 + (g0 + gn) * D, :].rearrange(
                    "(b p) e -> p b e", p=D)
                nc.sync.dma_start(dst, st[:, :gn * D].rearrange(
                    "p (b e) -> p b e", e=D))

        def lstm_side(t_lo, t_hi, width, x_terms, h_sb, c_sb, bias):
            for t in range(t_lo, t_hi):
                a, b2 = t * width, (t + 1) * width
                pg = [bank.tile(shape=[D, 4 * D], dtype=F32,
                                name="bk")[:, :width] for g in range(4)]
                for g in range(4):
                    for qi, (w_sb, rhs) in enumerate(x_terms):
                        nc.tensor.matmul(
                            pg[g][:],
                            w_sb[:, g * D:(g + 1) * D],
                            rhs[:, a:b2],
                            start=(qi == 0), stop=(qi == len(x_terms) - 1))
                nc.scalar.activation(pg[0][:], pg[0][:], AF.Sigmoid,
                                     bias=blc_or(bias, 0))
                nc.scalar.activation(pg[1][:], pg[1][:], AF.Sigmoid,
                                     bias=blc_or(bias, 1))
                tg = tmp.tile(shape=[D, width], dtype=F32, name="tg")
                nc.scalar.activation(tg[:], pg[2][:], AF.Tanh,
                                     bias=blc_or(bias, 2))
                nc.scalar.activation(pg[3][:], pg[3][:], AF.Sigmoid,
                                     bias=blc_or(bias, 3))
                t1 = tmp.tile(shape=[D, width], dtype=F32, name="t1")
                nc.vector.tensor_tensor(t1[:], pg[0][:], tg[:], op=OP.mult)
                nc.vector.tensor_tensor(c_sb[:, a:b2], c_sb[:, a:b2],
                                        pg[1][:], op=OP.mult)
                nc.vector.tensor_tensor(c_sb[:, a:b2], c_sb[:, a:b2], t1[:],
                                        op=OP.add)
                t2 = tmp.tile(shape=[D, width], dtype=F32, name="t2")
                nc.scalar.activation(t2[:], c_sb[:, a:b2], AF.Tanh)
                nc.vector.tensor_tensor(h_sb[:, a:b2],
                                        pg[3][:], t2[:], op=OP.mult)

        def blc_or(bias, g):
            return bias[:, g:g + 1]

        for it in range(num_iters):
            src_tab = x_tok if it == 0 else h_l_tab

            # --- direction 1 + clause LSTM, software-pipelined halves ---
            HC = CPAD // 2
            for a in (0, HC):
                gather_tr(src_tab, d1_idx, union, a, 0 * CPAD + a, HC, True)
                gather_tr(src_tab, d1_idx, union, a, 1 * CPAD + a, HC, False)
                gather_tr(src_tab, d1_idx, union, a, 2 * CPAD + a, HC, False)
                lstm_side(a // CW, (a + HC) // CW, CW,
                          [(w2, h_c), (w1, msg_c)], h_c, c_c, blc)
                transpose_to_table(h_c, a // D, (a + HC) // D, h_c_tab, 1)

            # --- direction 2 + flip + literal LSTM, pipelined halves ---
            HL = LPAD // 2
            gather_tr(src_tab, flip_idx, union, LPAD, 0, LPAD, True)
            for lo in (0, HL):
                hi = lo + HL
                first = True
                off = 0
                for n in d2_plane_sizes:
                    s0, s1 = lo, min(n, hi)
                    if s1 > s0:
                        gather_tr(h_c_tab, d2_idx, union, s0, off + s0,
                                  s1 - s0, first)
                    first = False
                    off += n
                lstm_side(lo // LW, hi // LW, LW,
                          [(w5, h_l), (w4, flip_b), (w3, msg_l)],
                          h_l, c_l, bcl)
                if it < num_iters - 1:
                    transpose_to_table(h_l, lo // D, hi // D, h_l_tab, 1)


        # readout: votes = out_w . h_l + out_b
        for t in range(NTILE_L):
            a, b2 = t * LW, (t + 1) * LW
            pv = bank.tile(shape=[D, 4 * D], dtype=F32, name="bk")[0:1, :LW]
            vst = stg.tile(shape=[1, LW], dtype=F32, name="vst")
            nc.tensor.matmul(pv[:], owc[:],
                             h_l[:, a:b2],
                             start=True, stop=True)
            nc.scalar.activation(vst[:], pv[:], AF.Identity,
                                 bias=obs[:, 0:1])
            nc.sync.dma_start(votes_out[:, a:b2], vst[:])
        nc.sync.dma_start(hl_out, h_l[:])

        # vote_reduced: indicator matmul over token-major h_l blocks
        pvr = psv.tile(shape=[4, D], dtype=F32, name="pvr")
        for t in range(LPAD // D):
            pt = bank.tile(shape=[D, 4 * D], dtype=F32, name="bk")[:, :D]
            st = stg.tile(shape=[D, D], dtype=F32, name="stv")
            nc.tensor.transpose(pt[:], h_l[:, t * D:(t + 1) * D], idm[:])
            nc.vector.tensor_copy(st[:], pt[:])
            nc.tensor.matmul(pvr[:], inds[:, t * 4:(t + 1) * 4], st[:],
                             start=(t == 0), stop=(t == LPAD // D - 1),
                             skip_group_check=True)
        nc.vector.tensor_tensor(svr[:], pvr[:], owb[:], op=OP.mult)
        nc.vector.tensor_reduce(rvr[:], svr[:], mybir.AxisListType.X, OP.add)
        nc.scalar.activation(vr_sb[:], rvr[:], AF.Identity,
                             bias=ob4[:, 0:1], scale=1.0 / NL_G)
        nc.sync.dma_start(vr_out, vr_sb[:])

    nc.compile()
    return nc


def _preprocess(inputs):
    """Slice per-core, degree-sort literals, build index planes + tables."""
    lit_idx = np.asarray(inputs["lit_idx"])
    clause_idx = np.asarray(inputs["clause_idx"])
    flip_perm = np.asarray(inputs["flip_perm"])
    x_unk = np.asarray(inputs["x_unk"], np.float32)

    order = np.argsort(clause_idx, kind="stable")
    lit_by_clause = lit_idx[order].reshape(B * NC_G, K)

    cores = []
    for c in range(N_CORES):
        l0, c0 = c * NLc, c * NCc
        lc = lit_by_clause[c0:c0 + NCc] - l0          # [NCc, K] in [0, NLc)
        deg = np.bincount(lc.reshape(-1), minlength=NLc)
        perm = np.argsort(-deg, kind="stable")        # rank -> orig literal
        rank_of = np.empty(NLc, np.int64)
        rank_of[perm] = np.arange(NLc)
        d1 = np.zeros((K, CPAD), np.int64)
        d1[:, :NCc] = (rank_of[lc] + 1).T
        sorted_deg = deg[perm]
        eorder = np.argsort(rank_of[lc.reshape(-1)], kind="stable")
        cl_of_edge = np.repeat(np.arange(NCc), K)[eorder]
        starts = np.zeros(NLc + 1, np.int64)
        np.cumsum(sorted_deg, out=starts[1:])
        fl = flip_perm[l0:l0 + NLc] - l0
        fidx = np.zeros(LPAD, np.int64)
        fidx[:NLc] = rank_of[fl[perm]] + 1
        cores.append(dict(perm=perm, rank_of=rank_of, sorted_deg=sorted_deg,
                          cl_of_edge=cl_of_edge, starts=starts, d1=d1,
                          fidx=fidx))

    max_deg = int(max(co["sorted_deg"][0] for co in cores))
    plane_sizes = [LPAD]
    for p in range(1, max_deg):
        n = max(int((co["sorted_deg"] > p).sum()) for co in cores)
        plane_sizes.append(min(-(-n // 128) * 128, LPAD))

    per_core = []
    for c, co in enumerate(cores):
        d2 = np.zeros((sum(plane_sizes),), np.int64)
        off = 0
        for p, npl in enumerate(plane_sizes):
            nreal = int((co["sorted_deg"] > p).sum())
            idxs = co["cl_of_edge"][co["starts"][:nreal] + p] + 1
            d2[off:off + nreal] = idxs
            off += npl
        x_core = x_unk[c * NLc:(c + 1) * NLc][co["perm"]]
        xl_feat = np.zeros((D, LPAD), np.float32)
        xl_feat[:, :NLc] = x_core.T
        x_tok = np.zeros((LT, D), np.float32)
        x_tok[1:1 + NLc] = x_core
        indicator = np.zeros((LPAD, 4), np.float32)
        g_of = co["perm"] // NL_G
        indicator[np.arange(NLc), g_of] = 1.0
        ind_sb = np.zeros((D, (LPAD // D) * 4), np.float32)
        for t in range(LPAD // D):
            ind_sb[:, t * 4:(t + 1) * 4] = indicator[t * D:(t + 1) * D]
        per_core.append(dict(
            d1_idx=_fmt_idx(co["d1"].reshape(-1)),
            d2_idx=_fmt_idx(d2),
            flip_idx=_fmt_idx(co["fidx"]),
            xl_feat=xl_feat, x_tok=x_tok, ind=ind_sb,
            perm=co["perm"], rank_of=co["rank_of"]))
    return per_core, tuple(plane_sizes)


def kernel(**inputs):
    num_iters = int(inputs["num_iters"])
    per_core, plane_sizes = _preprocess(inputs)

    f32 = lambda k: np.ascontiguousarray(np.asarray(inputs[k], np.float32))
    out_w = f32("out_w").reshape(D, 1)
    out_b = float(np.asarray(inputs["out_b"]).reshape(-1)[0])
    w_ih_cl = f32("Wih_cl").T.copy()

    shared = dict(
        hc0=(f32("C_w") + f32("C_b")).reshape(D, 1),
        w_ih_lc=f32("Wih_lc").T.copy(),
        w_hh_lc=f32("Whh_lc").T.copy(),
        w_ih_cl_m=np.ascontiguousarray(w_ih_cl[:D]),
        w_ih_cl_f=np.ascontiguousarray(w_ih_cl[D:]),
        w_hh_cl=f32("Whh_cl").T.copy(),
        b_lc=(f32("bih_lc") + f32("bhh_lc")).reshape(4, D).T.copy(),
        b_cl=(f32("bih_cl") + f32("bhh_cl")).reshape(4, D).T.copy(),
        out_w_b=np.tile(out_w.reshape(1, D), (4, 1)).copy(),
        out_w_col=out_w.copy(),
        out_b=np.full((1, 1), out_b, np.float32),
        out_b4=np.full((4, 1), out_b, np.float32),
        identity=np.eye(D, dtype=np.float32),
    )

    key = (num_iters, plane_sizes)
    if key not in _prog_cache:
        _prog_cache[key] = _build_program(num_iters, plane_sizes)
    nc = _prog_cache[key]

    in_maps = []
    for c in range(N_CORES):
        pc = per_core[c]
        m = dict(shared)
        m.update(xl_feat=pc["xl_feat"], x_tok=pc["x_tok"], ind=pc["ind"],
                 d1_idx=pc["d1_idx"], d2_idx=pc["d2_idx"],
                 flip_idx=pc["flip_idx"])
        in_maps.append(m)

    res = bass_utils.run_bass_kernel_spmd(nc, in_maps,
                                          core_ids=list(range(N_CORES)))

    h_l = np.empty((B * NL_G, D), np.float32)
    votes = np.empty((B * NL_G, 1), np.float32)
    vote_reduced = np.empty((B, 1), np.float32)
    for c in range(N_CORES):
        r = res.results[c]
        pc = per_core[c]
        h_l[c * NLc:(c + 1) * NLc] = r["hl_out"][:, :NLc].T[pc["rank_of"]]
        votes[c * NLc:(c + 1) * NLc] = \
            r["votes_out"][0, :NLc].reshape(-1, 1)[pc["rank_of"]]
        vote_reduced[c * GPC:(c + 1) * GPC] = r["vr_out"]
    return vote_reduced, votes, h_l
